# revision 8
# baseline (speedup 1.0000x reference)
"""Trainium2 Bass kernel for a char-CNN (embed lookup + conv1d(K=5,pad=2) + bias + maxpool).

Math: out[n, f] = max_w ( b[f] + sum_k sum_d  E[ids[n, w+k-2], d] * Wc[f, d, k] )

Strategy (pure data-parallel over 8 cores, 4096 tokens each):
  * Host-side constant folding (weights only): G[k][v, f] = sum_d E[v, d] * Wc[f, d, k].
    The embedding+conv collapses to y[n,:,w] = sum_k G[k][ids[n,w+k-2], :] + b.
  * On device, table lookup runs on the TensorEngine as one-hot matmuls with
    contraction over the vocab (96) plus a constant-ones row that carries the bias:
      - broadcast ids across partitions with K=1 ones-matmuls (two concurrent
        row-groups via base partitions 0/32)                  -> psum [96, cols]
      - one-hot = is_equal(bcast, iota_per_partition) on VectorE, written f32r
        into a padded [vocab+1, tokens, W+4] layout so both the is_equal write
        and the maxpool reads are dense, and the 5 tap reads are shifted views
        with token boundaries seeing zeros
      - 5 taps x 2 precision splits PSUM-accumulated against G tables stored as
        fp16 hi + lo (hi+lo recovers ~22 mantissa bits); fp16 weights padded to
        128 columns enable fast-weight-load so LDWEIGHTS hides under the matmul
      - reduce_max over the 16 positions runs on the (otherwise idle) GpSimd
        engine so the VectorE only produces one-hots
  * The broadcast/one-hot for unit u+1 is emitted before unit u's taps so the
    in-order PE queue never stalls on the VectorE.
  * Output is produced as [group, F, 512] per core; host transposes/concats.
"""

import numpy as np

import concourse.bass as bass
import concourse.bacc as bacc
import concourse.mybir as mybir
from concourse.tile import TileContext
from concourse.bass_utils import run_bass_kernel_spmd

# Problem shapes (hardcoded per contract)
N, W = 32768, 16
VOCAB, D, F, K = 96, 100, 100, 5
N_CORES = 8
NSH = N // N_CORES            # tokens per core = 4096
UNIT = 64                     # tokens per pipeline unit (=> 1024 one-hot cols)
NUNIT = NSH // UNIT           # 64
GROUP = 512                   # tokens per ids DMA
NGROUP = NSH // GROUP         # 8
UPG = GROUP // UNIT           # units per group = 8
VP = VOCAB + 1                # 96 vocab rows + 1 ones row (bias)
WP = W + 4                    # padded char positions per token
FP = 128                      # F padded to 128 weight columns (enables FWL)

f16 = mybir.dt.float16
f32 = mybir.dt.float32
f32r = mybir.dt.float32r
i32 = mybir.dt.int32


def build_nc():
    nc = bacc.Bacc("TRN2", target_bir_lowering=False)

    ids_d = nc.dram_tensor("ids", [NSH, W], i32, kind="ExternalInput")
    # G split tables: [v, (split s, tap k), f_padded]  s=0 -> fp16(G), s=1 -> fp16(G - hi)
    gtab_d = nc.dram_tensor("gtab", [VP, 2 * K, FP], f16, kind="ExternalInput")
    iota_d = nc.dram_tensor("iota", [VOCAB, 1], f32, kind="ExternalInput")
    ones_d = nc.dram_tensor("ones", [33, VOCAB], f32r, kind="ExternalInput")
    oones_d = nc.dram_tensor("oones", [1, UNIT * WP], f16, kind="ExternalInput")
    out_d = nc.dram_tensor("out", [NGROUP, F, GROUP], f32, kind="ExternalOutput")

    with TileContext(nc) as tc:
        with (
            tc.tile_pool(name="consts", bufs=1) as consts,
            tc.tile_pool(name="outp", bufs=2) as outp,
            tc.tile_pool(name="idsp", bufs=3) as idsp,
            tc.tile_pool(name="psA", bufs=2, space="PSUM") as psA,
            tc.tile_pool(name="psB", bufs=2, space="PSUM") as psB,
        ):
            iota_t = consts.tile([VOCAB, 1], f32)
            nc.gpsimd.dma_start(out=iota_t, in_=iota_d[:, :])
            # touch DVE/GpSimd with their steady-state opcodes early: absorbs
            # each engine's first-dispatch latency during the init phase.
            dve_warm = consts.tile([VOCAB, 2], f32, tag="dve_warm")
            nc.vector.tensor_scalar(
                out=dve_warm[:, 0:1],
                in0=iota_t[:, :],
                scalar1=iota_t[:, 0:1],
                scalar2=None,
                op0=mybir.AluOpType.is_equal,
            )
            nc.vector.reduce_max(
                out=dve_warm[:, 1:2],
                in_=iota_t[:, :],
                axis=mybir.AxisListType.X,
            )
            ones_t = consts.tile([33, VOCAB], f32r)
            nc.gpsimd.dma_start(out=ones_t, in_=ones_d[:, :])

            # Two persistent one-hot tiles, padded layout [VP, UNIT, W+4]:
            # char position w at column w+2, pad columns {0,1,18,19} stay zero,
            # row 96 constant 1.0 (bias row, consumed only by center tap).
            o_tiles = []
            for j in range(2):
                ot = consts.tile([VP, WP, UNIT], f16, tag=f"onehot{j}")
                # zero everything once; is_equal rewrites the vocab rows' real
                # positions every unit, pads stay zero forever.
                nc.vector.memset(ot[0:VOCAB, :, :], 0.0)
                nc.gpsimd.dma_start(
                    out=ot[VOCAB : VOCAB + 1, :, :].rearrange("v w t -> v (w t)"),
                    in_=oones_d[:, :],
                )
                o_tiles.append(ot)

            ids_tiles = {}

            def load_ids(g):
                idst = idsp.tile([33, GROUP * W // 2], f32r, tag="ids")
                v = ids_d[g * GROUP : (g + 1) * GROUP, :].rearrange(
                    "(b a t) w -> b a (t w)", a=2, t=32
                )
                nc.gpsimd.dma_start(out=idst[0:1, :], in_=v[:, 0, :])
                nc.gpsimd.dma_start(out=idst[32:33, :], in_=v[:, 1, :])
                ids_tiles[g] = idst

            def bcast(u):
                # broadcast ids across 96 partitions (K=1 matmul) + one-hot
                g, uu = divmod(u, UPG)
                idst = ids_tiles[g]
                bc = psA.tile([VOCAB, UNIT, W], f32, tag="bcast")
                for h in range(2):
                    p0 = 32 * h
                    nc.tensor.matmul(
                        bc[:, h * 32 : (h + 1) * 32, :],
                        ones_t[p0 : p0 + 1, :],
                        idst[p0 : p0 + 1, uu * 512 : (uu + 1) * 512],
                        start=True,
                        stop=True,
                    )
                # one-hot: O[v, t, w+2] = (ids[t, w] == v), dense write
                # (must run on DVE: GPSIMD cannot read PSUM)
                o_t = o_tiles[u % 2]
                nc.vector.tensor_scalar(
                    out=o_t[0:VOCAB, 2 : 2 + W, :].rearrange("v p t -> v t p"),
                    in0=bc[:, :, :],
                    scalar1=iota_t[:, 0:1],
                    scalar2=None,
                    op0=mybir.AluOpType.is_equal,
                )

            gtab = consts.tile([VP, 2 * K, FP], f16)
            nc.gpsimd.dma_start(
                out=gtab.rearrange("v s f -> v (s f)"),
                in_=gtab_d.rearrange("v s f -> v (s f)"),
            )

            # PE warmup: tiny matmuls keep the HAM activity window busy while
            # the init DMAs land, so real matmuls start at full clock.
            warm = psA.tile([1, 1], f32, tag="bcast")
            for _ in range(48):
                nc.tensor.matmul(
                    warm[0:1, 0:1],
                    iota_t[0:1, 0:1],
                    iota_t[0:1, 0:1],
                    start=True,
                    stop=True,
                )

            load_ids(0)
            load_ids(1)
            bcast(0)
            out_sb = None
            for u in range(NUNIT):
                g, uu = divmod(u, UPG)
                if uu == 0:
                    out_sb = outp.tile([F, GROUP], f32, tag="osb")
                    if g + 2 < NGROUP:
                        load_ids(g + 2)
                # emit next unit's bcast+one-hot BEFORE this unit's taps so the
                # in-order PE queue never stalls waiting on the DVE is_equal.
                if u + 1 < NUNIT:
                    bcast(u + 1)

                o_t = o_tiles[u % 2]
                # 5 taps x 2 precision splits, PSUM-accumulated (N=512 each)
                ys = [psB.tile([FP, W, 32], f32, tag=f"y{h}", name=f"y{h}") for h in range(2)]
                first = True
                for s in range(2):
                    for k in range(K):
                        for h in range(2):
                            nc.tensor.matmul(
                                ys[h][:, :, :],
                                gtab[:, s * K + k, :],
                                o_t[:, k : k + W, h * 32 : (h + 1) * 32],
                                start=first,
                                stop=(s == 1 and k == K - 1),
                                skip_group_check=True,
                            )
                        first = False

                # max over the 16 char positions (dense innermost reduce)
                for h in range(2):
                    nc.vector.reduce_max(
                        out=out_sb[:, uu * UNIT + h * 32 : uu * UNIT + (h + 1) * 32],
                        in_=ys[h][0:F, :, :].rearrange("f w t -> f t w"),
                        axis=mybir.AxisListType.X,
                    )

                if uu == UPG - 1:
                    # stream this group's result out to DRAM (contiguous block)
                    nc.sync.dma_start(out=out_d[g, :, :], in_=out_sb[:, :])

    nc.compile()
    return nc


def make_consts(embed_table, conv_w, conv_b):
    # G[k][v, f] = sum_d E[v, d] * Wc[f, d, k] in float64, split hi/lo fp16
    G = np.einsum(
        "vd,fdk->kvf", embed_table.astype(np.float64), conv_w.astype(np.float64)
    )
    Gf = np.zeros((K, VP, F), np.float64)
    Gf[:, 0:VOCAB, :] = G
    Gf[2, VOCAB, :] = conv_b.astype(np.float64)  # bias rides center tap
    hi = Gf.astype(np.float32).astype(np.float16)
    lo = (Gf - hi.astype(np.float64)).astype(np.float32).astype(np.float16)
    gtab = np.zeros((VP, 2 * K, FP), np.float16)
    gtab[:, 0:K, 0:F] = np.transpose(hi, (1, 0, 2))
    gtab[:, K : 2 * K, 0:F] = np.transpose(lo, (1, 0, 2))
    iota = np.arange(VOCAB, dtype=np.float32).reshape(VOCAB, 1)
    ones = np.zeros((33, VOCAB), np.float32)
    ones[0, :] = 1.0
    ones[32, :] = 1.0
    oones = np.ones((1, UNIT * WP), np.float16)
    return gtab, iota, ones, oones


_NC_CACHE = {}

# Test-harness knobs (ignored by normal kernel() use)
TRACE = False
LAST_RESULT = None


def kernel(char_ids, embed_table, conv_w, conv_b):
    global LAST_RESULT
    char_ids = np.asarray(char_ids)
    gtab, iota, ones, oones = make_consts(
        np.asarray(embed_table), np.asarray(conv_w), np.asarray(conv_b)
    )

    if "nc" not in _NC_CACHE:
        _NC_CACHE["nc"] = build_nc()
    nc = _NC_CACHE["nc"]

    in_maps = []
    for c in range(N_CORES):
        shard = np.ascontiguousarray(char_ids[c * NSH : (c + 1) * NSH])
        in_maps.append(
            {"ids": shard, "gtab": gtab, "iota": iota, "ones": ones, "oones": oones}
        )

    kwargs = {}
    if TRACE:
        kwargs = dict(trace=True, trace_cores=list(range(N_CORES)))
    res = run_bass_kernel_spmd(nc, in_maps, core_ids=list(range(N_CORES)), **kwargs)
    LAST_RESULT = res

    out = np.empty((N, F), np.float32)
    for c in range(N_CORES):
        o = res.results[c]["out"]  # [NGROUP, F, GROUP]
        out[c * NSH : (c + 1) * NSH] = o.transpose(0, 2, 1).reshape(NSH, F)
    return out


# revision 9
# speedup vs baseline: 1.1834x; 1.1834x over previous
"""Trainium2 Bass kernel for a char-CNN (embed lookup + conv1d(K=5,pad=2) + bias + maxpool).

Math: out[n, f] = max_w ( b[f] + sum_k sum_d  E[ids[n, w+k-2], d] * Wc[f, d, k] )

Strategy (pure data-parallel over 8 cores, 4096 tokens each):
  * Host-side constant folding (weights only): G[k][v, f] = sum_d E[v, d] * Wc[f, d, k].
    The embedding+conv collapses to y[n,:,w] = sum_k G[k][ids[n,w+k-2], :] + b.
  * On device, table lookup runs on the TensorEngine as one-hot matmuls with
    contraction over the vocab (96) plus a constant-ones row that carries the bias:
      - ids (bf16, exact for 0..95) broadcast across 96 partitions by the
        otherwise-idle GpSimd engine's partition_broadcast custom instruction,
        keeping the TensorEngine free for tap matmuls only
      - one-hot = is_equal(bcast, iota_per_partition) on VectorE, written fp16
        into a padded [vocab+1, tokens, W+4] layout (dense writes); the 5 tap
        reads are shifted views with token boundaries seeing zeros
      - 5 taps x 2 precision splits PSUM-accumulated against G tables stored as
        fp16 hi + lo (hi+lo recovers ~22 mantissa bits); fp16 weights padded to
        128 columns keep LDWEIGHTS (fast-weight-load) hidden under the matmuls
      - reduce_max over the 16 positions on VectorE (PSUM is DVE-only)
  * The broadcast/one-hot for unit u+1 is emitted before unit u's taps so the
    in-order PE queue never stalls on the VectorE.
  * Output is produced as [group, F, 512] per core; host transposes/concats.
"""

import numpy as np

import concourse.bass as bass
import concourse.bacc as bacc
import concourse.mybir as mybir
from concourse.tile import TileContext
from concourse.bass_utils import run_bass_kernel_spmd

# Problem shapes (hardcoded per contract)
N, W = 32768, 16
VOCAB, D, F, K = 96, 100, 100, 5
N_CORES = 8
NSH = N // N_CORES            # tokens per core = 4096
UNIT = 64                     # tokens per pipeline unit (=> 1024 one-hot cols)
NUNIT = NSH // UNIT           # 64
GROUP = 512                   # tokens per ids DMA
NGROUP = NSH // GROUP         # 8
UPG = GROUP // UNIT           # units per group = 8
VP = VOCAB + 1                # 96 vocab rows + 1 ones row (bias)
WP = W + 4                    # padded char positions per token
FP = 128                      # F padded to 128 weight columns (enables FWL)

bf16 = mybir.dt.bfloat16
f16 = mybir.dt.float16
f32 = mybir.dt.float32
i32 = mybir.dt.int32


def build_nc():
    nc = bacc.Bacc("TRN2", target_bir_lowering=False)

    ids_d = nc.dram_tensor("ids", [NSH, W], i32, kind="ExternalInput")
    # G split tables: [v, (split s, tap k), f_padded]  s=0 -> fp16(G), s=1 -> fp16(G - hi)
    gtab_d = nc.dram_tensor("gtab", [VP, 2 * K, FP], f16, kind="ExternalInput")
    iota_d = nc.dram_tensor("iota", [VOCAB, 1], f32, kind="ExternalInput")
    oones_d = nc.dram_tensor("oones", [1, UNIT * WP], f16, kind="ExternalInput")
    out_d = nc.dram_tensor("out", [NGROUP, F, GROUP], f32, kind="ExternalOutput")

    with TileContext(nc) as tc:
        with (
            tc.tile_pool(name="consts", bufs=1) as consts,
            tc.tile_pool(name="outp", bufs=2) as outp,
            tc.tile_pool(name="idsp", bufs=3) as idsp,
            tc.tile_pool(name="bcp", bufs=2) as bcp,
            tc.tile_pool(name="psA", bufs=1, space="PSUM") as psA,
            tc.tile_pool(name="psB", bufs=3, space="PSUM") as psB,
        ):
            iota_t = consts.tile([VOCAB, 1], f32)
            nc.gpsimd.dma_start(out=iota_t, in_=iota_d[:, :])
            # touch DVE with its steady-state opcodes early: absorbs the
            # engine's first-dispatch latency during the init phase.
            dve_warm = consts.tile([VOCAB, 2], f32, tag="dve_warm")
            nc.vector.tensor_scalar(
                out=dve_warm[:, 0:1],
                in0=iota_t[:, :],
                scalar1=iota_t[:, 0:1],
                scalar2=None,
                op0=mybir.AluOpType.is_equal,
            )
            nc.vector.reduce_max(
                out=dve_warm[:, 1:2],
                in_=iota_t[:, :],
                axis=mybir.AxisListType.X,
            )

            # Two persistent one-hot tiles, padded layout [VP, UNIT, W+4]:
            # char position w at column w+2, pad columns {0,1,18,19} stay zero,
            # row 96 constant 1.0 (bias row, consumed only by center tap).
            o_tiles = []
            for j in range(2):
                ot = consts.tile([VP, UNIT, WP], f16, tag=f"onehot{j}")
                # zero everything once; is_equal rewrites the vocab rows' real
                # positions every unit, pads stay zero forever.
                nc.vector.memset(ot[0:VOCAB, :, :], 0.0)
                nc.gpsimd.dma_start(
                    out=ot[VOCAB : VOCAB + 1, :, :].rearrange("v t w -> v (t w)"),
                    in_=oones_d[:, :],
                )
                o_tiles.append(ot)

            ids_tiles = {}

            def load_ids(g):
                # all of a group's ids in partition 0, converted to bf16
                # (0..95 are exact in bf16)
                idst = idsp.tile([1, GROUP * W], bf16, tag="ids")
                nc.gpsimd.dma_start(
                    out=idst[0:1, :],
                    in_=ids_d[g * GROUP : (g + 1) * GROUP, :].rearrange(
                        "t w -> (t w)"
                    ).unsqueeze(0),
                )
                ids_tiles[g] = idst

            def bcast(u):
                # broadcast ids across 96 partitions on GpSimd + one-hot on DVE
                g, uu = divmod(u, UPG)
                idst = ids_tiles[g]
                bcz = bcp.tile([VOCAB, UNIT, W], bf16, tag="bcast")
                nc.gpsimd.partition_broadcast(
                    out_ap=bcz.rearrange("v t w -> v (t w)"),
                    in_ap=idst[0:1, uu * (UNIT * W) : (uu + 1) * (UNIT * W)],
                    channels=VOCAB,
                )
                # one-hot: O[v, t, w+2] = (ids[t, w] == v), dense write
                o_t = o_tiles[u % 2]
                nc.vector.tensor_scalar(
                    out=o_t[0:VOCAB, :, 2 : 2 + W],
                    in0=bcz[:, :, :],
                    scalar1=iota_t[:, 0:1],
                    scalar2=None,
                    op0=mybir.AluOpType.is_equal,
                )

            gtab = consts.tile([VP, 2 * K, FP], f16)
            nc.gpsimd.dma_start(
                out=gtab.rearrange("v s f -> v (s f)"),
                in_=gtab_d.rearrange("v s f -> v (s f)"),
            )

            # PE warmup: tiny matmuls keep the HAM activity window busy while
            # the init DMAs land, so real matmuls start at full clock.
            warm = psA.tile([1, 1], f32, tag="warm")
            for _ in range(48):
                nc.tensor.matmul(
                    warm[0:1, 0:1],
                    iota_t[0:1, 0:1],
                    iota_t[0:1, 0:1],
                    start=True,
                    stop=True,
                )

            load_ids(0)
            load_ids(1)
            bcast(0)
            out_sb = None
            for u in range(NUNIT):
                g, uu = divmod(u, UPG)
                if uu == 0:
                    out_sb = outp.tile([F, GROUP], f32, tag="osb")
                    if g + 2 < NGROUP:
                        load_ids(g + 2)
                # emit next unit's bcast+one-hot BEFORE this unit's taps so the
                # in-order PE queue never stalls waiting on the DVE is_equal.
                if u + 1 < NUNIT:
                    bcast(u + 1)

                o_t = o_tiles[u % 2]
                # 5 taps x 2 precision splits, PSUM-accumulated (N=512 each)
                ys = [psB.tile([FP, 32, W], f32, tag=f"y{h}", name=f"y{h}") for h in range(2)]
                first = True
                for s in range(2):
                    for k in range(K):
                        for h in range(2):
                            nc.tensor.matmul(
                                ys[h][:, :, :],
                                gtab[:, s * K + k, :],
                                o_t[:, h * 32 : (h + 1) * 32, k : k + W],
                                start=first,
                                stop=(s == 1 and k == K - 1),
                                skip_group_check=True,
                            )
                        first = False

                # max over the 16 char positions (dense innermost reduce)
                for h in range(2):
                    nc.vector.reduce_max(
                        out=out_sb[:, uu * UNIT + h * 32 : uu * UNIT + (h + 1) * 32],
                        in_=ys[h][0:F, :, :],
                        axis=mybir.AxisListType.X,
                    )

                if uu == UPG - 1:
                    # stream this group's result out to DRAM (contiguous block)
                    nc.sync.dma_start(out=out_d[g, :, :], in_=out_sb[:, :])

    nc.compile()
    return nc


def make_consts(embed_table, conv_w, conv_b):
    # G[k][v, f] = sum_d E[v, d] * Wc[f, d, k] in float64, split hi/lo fp16
    G = np.einsum(
        "vd,fdk->kvf", embed_table.astype(np.float64), conv_w.astype(np.float64)
    )
    Gf = np.zeros((K, VP, F), np.float64)
    Gf[:, 0:VOCAB, :] = G
    Gf[2, VOCAB, :] = conv_b.astype(np.float64)  # bias rides center tap
    hi = Gf.astype(np.float32).astype(np.float16)
    lo = (Gf - hi.astype(np.float64)).astype(np.float32).astype(np.float16)
    gtab = np.zeros((VP, 2 * K, FP), np.float16)
    gtab[:, 0:K, 0:F] = np.transpose(hi, (1, 0, 2))
    gtab[:, K : 2 * K, 0:F] = np.transpose(lo, (1, 0, 2))
    iota = np.arange(VOCAB, dtype=np.float32).reshape(VOCAB, 1)
    oones = np.ones((1, UNIT * WP), np.float16)
    return gtab, iota, oones


_NC_CACHE = {}

# Test-harness knobs (ignored by normal kernel() use)
TRACE = False
LAST_RESULT = None


def kernel(char_ids, embed_table, conv_w, conv_b):
    global LAST_RESULT
    char_ids = np.asarray(char_ids)
    gtab, iota, oones = make_consts(
        np.asarray(embed_table), np.asarray(conv_w), np.asarray(conv_b)
    )

    if "nc" not in _NC_CACHE:
        _NC_CACHE["nc"] = build_nc()
    nc = _NC_CACHE["nc"]

    in_maps = []
    for c in range(N_CORES):
        shard = np.ascontiguousarray(char_ids[c * NSH : (c + 1) * NSH])
        in_maps.append({"ids": shard, "gtab": gtab, "iota": iota, "oones": oones})

    kwargs = {}
    if TRACE:
        kwargs = dict(trace=True, trace_cores=list(range(N_CORES)))
    res = run_bass_kernel_spmd(nc, in_maps, core_ids=list(range(N_CORES)), **kwargs)
    LAST_RESULT = res

    out = np.empty((N, F), np.float32)
    for c in range(N_CORES):
        o = res.results[c]["out"]  # [NGROUP, F, GROUP]
        out[c * NSH : (c + 1) * NSH] = o.transpose(0, 2, 1).reshape(NSH, F)
    return out


# revision 10
# speedup vs baseline: 1.2310x; 1.0402x over previous
"""Trainium2 Bass kernel for a char-CNN (embed lookup + conv1d(K=5,pad=2) + bias + maxpool).

Math: out[n, f] = max_w ( b[f] + sum_k sum_d  E[ids[n, w+k-2], d] * Wc[f, d, k] )

Strategy (pure data-parallel over 8 cores, 4096 tokens each):
  * Host-side constant folding (weights only): G[k][v, f] = sum_d E[v, d] * Wc[f, d, k].
    The embedding+conv collapses to y[n,:,w] = sum_k G[k][ids[n,w+k-2], :] + b.
  * On device, table lookup runs on the TensorEngine as one-hot matmuls with
    contraction over the vocab (96) plus a constant-ones row that carries the bias:
      - ids (bf16, exact for 0..95) broadcast across 96 partitions by the
        otherwise-idle GpSimd engine's partition_broadcast custom instruction,
        keeping the TensorEngine free for tap matmuls only
      - one-hot = is_equal(bcast, iota_per_partition) on VectorE, written fp16
        into a padded [vocab+1, tokens, W+4] layout (dense writes); the 5 tap
        reads are shifted views with token boundaries seeing zeros
      - 5 taps x 2 precision splits PSUM-accumulated against G tables stored as
        fp16 hi + lo (hi+lo recovers ~22 mantissa bits); fp16 weights padded to
        128 columns keep LDWEIGHTS (fast-weight-load) hidden under the matmuls
      - reduce_max over the 16 positions on VectorE (PSUM is DVE-only)
  * The broadcast/one-hot for unit u+1 is emitted before unit u's taps so the
    in-order PE queue never stalls on the VectorE.
  * Output is produced as [group, F, 512] per core; host transposes/concats.
"""

import numpy as np

import concourse.bass as bass
import concourse.bacc as bacc
import concourse.mybir as mybir
from concourse.tile import TileContext
from concourse.bass_utils import run_bass_kernel_spmd

# Problem shapes (hardcoded per contract)
N, W = 32768, 16
VOCAB, D, F, K = 96, 100, 100, 5
N_CORES = 8
NSH = N // N_CORES            # tokens per core = 4096
UNIT = 64                     # tokens per pipeline unit (=> 1024 one-hot cols)
NUNIT = NSH // UNIT           # 64
GROUP = 512                   # tokens per ids DMA
NGROUP = NSH // GROUP         # 8
UPG = GROUP // UNIT           # units per group = 8
VP = VOCAB + 1                # 96 vocab rows + 1 ones row (bias)
WP = W + 4                    # padded char positions per token
FP = 128                      # F padded to 128 weight columns (enables FWL)

bf16 = mybir.dt.bfloat16
f16 = mybir.dt.float16
f32 = mybir.dt.float32
i32 = mybir.dt.int32


def build_nc():
    nc = bacc.Bacc("TRN2", target_bir_lowering=False)

    ids_d = nc.dram_tensor("ids", [NSH, W], i32, kind="ExternalInput")
    # G split tables: [v, (split s, tap k), f_padded]  s=0 -> fp16(G), s=1 -> fp16(G - hi)
    gtab_d = nc.dram_tensor("gtab", [VP, 2 * K, FP], f16, kind="ExternalInput")
    iota_d = nc.dram_tensor("iota", [VOCAB, 1], f32, kind="ExternalInput")
    oones_d = nc.dram_tensor("oones", [1, UNIT * WP], f16, kind="ExternalInput")
    out_d = nc.dram_tensor("out", [NGROUP, F, GROUP], f32, kind="ExternalOutput")

    with TileContext(nc) as tc:
        with (
            tc.tile_pool(name="consts", bufs=1) as consts,
            tc.tile_pool(name="outp", bufs=2) as outp,
            tc.tile_pool(name="idsp", bufs=3) as idsp,
            tc.tile_pool(name="bcp", bufs=2) as bcp,
            tc.tile_pool(name="psA", bufs=1, space="PSUM") as psA,
            tc.tile_pool(name="psB", bufs=3, space="PSUM") as psB,
        ):
            iota_t = consts.tile([VOCAB, 1], f32)
            nc.gpsimd.dma_start(out=iota_t, in_=iota_d[:, :])
            # touch DVE with its steady-state opcodes early: absorbs the
            # engine's first-dispatch latency during the init phase.
            dve_warm = consts.tile([VOCAB, 2], f32, tag="dve_warm")
            nc.vector.tensor_scalar(
                out=dve_warm[:, 0:1],
                in0=iota_t[:, :],
                scalar1=iota_t[:, 0:1],
                scalar2=None,
                op0=mybir.AluOpType.is_equal,
            )
            nc.vector.reduce_max(
                out=dve_warm[:, 1:2],
                in_=iota_t[:, :],
                axis=mybir.AxisListType.X,
            )

            # Two persistent one-hot tiles, padded layout [VP, UNIT, W+4]:
            # char position w at column w+2, pad columns {0,1,18,19} stay zero,
            # row 96 constant 1.0 (bias row, consumed only by center tap).
            o_tiles = []
            s_tiles = []
            for j in range(2):
                ot = consts.tile([VP, UNIT, WP], f16, tag=f"onehot{j}")
                # zero everything once; is_equal rewrites the vocab rows' real
                # positions every unit, pads stay zero forever.
                nc.vector.memset(ot[0:VOCAB, :, :], 0.0)
                nc.gpsimd.dma_start(
                    out=ot[VOCAB : VOCAB + 1, :, :].rearrange("v t w -> v (t w)"),
                    in_=oones_d[:, :],
                )
                o_tiles.append(ot)
                # shifted copy (w+1), maintained by the idle Scalar engine so
                # odd taps read 4-byte-aligned fp16 offsets
                st = consts.tile([VP, UNIT, WP], f16, tag=f"oshift{j}")
                nc.vector.memset(st[:, :, :], 0.0)
                s_tiles.append(st)

            ids_tiles = {}

            def load_ids(g):
                # all of a group's ids in partition 0, converted to bf16
                # (0..95 are exact in bf16)
                idst = idsp.tile([1, GROUP * W], bf16, tag="ids")
                nc.gpsimd.dma_start(
                    out=idst[0:1, :],
                    in_=ids_d[g * GROUP : (g + 1) * GROUP, :].rearrange(
                        "t w -> (t w)"
                    ).unsqueeze(0),
                )
                ids_tiles[g] = idst

            def bcast(u):
                # broadcast ids across 96 partitions on GpSimd + one-hot on DVE
                g, uu = divmod(u, UPG)
                idst = ids_tiles[g]
                bcz = bcp.tile([VOCAB, UNIT, W], bf16, tag="bcast")
                nc.gpsimd.partition_broadcast(
                    out_ap=bcz.rearrange("v t w -> v (t w)"),
                    in_ap=idst[0:1, uu * (UNIT * W) : (uu + 1) * (UNIT * W)],
                    channels=VOCAB,
                )
                # one-hot: O[v, t, w+2] = (ids[t, w] == v), dense write
                o_t = o_tiles[u % 2]
                nc.vector.tensor_scalar(
                    out=o_t[0:VOCAB, :, 2 : 2 + W],
                    in0=bcz[:, :, :],
                    scalar1=iota_t[:, 0:1],
                    scalar2=None,
                    op0=mybir.AluOpType.is_equal,
                )
                # shift-by-one copy for the odd taps (Scalar engine, off the
                # critical path)
                o_s = s_tiles[u % 2]
                nc.scalar.copy(
                    out=o_s[:, :, 0 : WP - 1],
                    in_=o_t[:, :, 1:WP],
                )

            gtab = consts.tile([VP, 2 * K, FP], f16)
            nc.gpsimd.dma_start(
                out=gtab.rearrange("v s f -> v (s f)"),
                in_=gtab_d.rearrange("v s f -> v (s f)"),
            )

            # PE warmup: tiny matmuls keep the HAM activity window busy while
            # the init DMAs land, so real matmuls start at full clock.
            warm = psA.tile([1, 1], f32, tag="warm")
            for _ in range(48):
                nc.tensor.matmul(
                    warm[0:1, 0:1],
                    iota_t[0:1, 0:1],
                    iota_t[0:1, 0:1],
                    start=True,
                    stop=True,
                )

            load_ids(0)
            load_ids(1)
            bcast(0)
            out_sb = None
            for u in range(NUNIT):
                g, uu = divmod(u, UPG)
                if uu == 0:
                    out_sb = outp.tile([F, GROUP], f32, tag="osb")
                    if g + 2 < NGROUP:
                        load_ids(g + 2)
                # emit next unit's bcast+one-hot BEFORE this unit's taps so the
                # in-order PE queue never stalls waiting on the DVE is_equal.
                if u + 1 < NUNIT:
                    bcast(u + 1)

                o_t = o_tiles[u % 2]
                o_s = s_tiles[u % 2]
                # 5 taps x 2 precision splits, PSUM-accumulated (N=512 each).
                # Odd taps read the shifted tile at even (4B-aligned) offsets.
                ys = [psB.tile([FP, 32, W], f32, tag=f"y{h}", name=f"y{h}") for h in range(2)]
                first = True
                for s in range(2):
                    for k in range(K):
                        src_t, kk = (o_t, k) if k % 2 == 0 else (o_s, k - 1)
                        for h in range(2):
                            nc.tensor.matmul(
                                ys[h][:, :, :],
                                gtab[:, s * K + k, :],
                                src_t[:, h * 32 : (h + 1) * 32, kk : kk + W],
                                start=first,
                                stop=(s == 1 and k == K - 1),
                                skip_group_check=True,
                            )
                        first = False

                # max over the 16 char positions (dense innermost reduce)
                for h in range(2):
                    nc.vector.reduce_max(
                        out=out_sb[:, uu * UNIT + h * 32 : uu * UNIT + (h + 1) * 32],
                        in_=ys[h][0:F, :, :],
                        axis=mybir.AxisListType.X,
                    )

                if uu == UPG - 1:
                    # stream this group's result out to DRAM (contiguous block)
                    nc.sync.dma_start(out=out_d[g, :, :], in_=out_sb[:, :])

    nc.compile()
    return nc


def make_consts(embed_table, conv_w, conv_b):
    # G[k][v, f] = sum_d E[v, d] * Wc[f, d, k] in float64, split hi/lo fp16
    G = np.einsum(
        "vd,fdk->kvf", embed_table.astype(np.float64), conv_w.astype(np.float64)
    )
    Gf = np.zeros((K, VP, F), np.float64)
    Gf[:, 0:VOCAB, :] = G
    Gf[2, VOCAB, :] = conv_b.astype(np.float64)  # bias rides center tap
    hi = Gf.astype(np.float32).astype(np.float16)
    lo = (Gf - hi.astype(np.float64)).astype(np.float32).astype(np.float16)
    gtab = np.zeros((VP, 2 * K, FP), np.float16)
    gtab[:, 0:K, 0:F] = np.transpose(hi, (1, 0, 2))
    gtab[:, K : 2 * K, 0:F] = np.transpose(lo, (1, 0, 2))
    iota = np.arange(VOCAB, dtype=np.float32).reshape(VOCAB, 1)
    oones = np.ones((1, UNIT * WP), np.float16)
    return gtab, iota, oones


_NC_CACHE = {}

# Test-harness knobs (ignored by normal kernel() use)
TRACE = False
LAST_RESULT = None


def kernel(char_ids, embed_table, conv_w, conv_b):
    global LAST_RESULT
    char_ids = np.asarray(char_ids)
    gtab, iota, oones = make_consts(
        np.asarray(embed_table), np.asarray(conv_w), np.asarray(conv_b)
    )

    if "nc" not in _NC_CACHE:
        _NC_CACHE["nc"] = build_nc()
    nc = _NC_CACHE["nc"]

    in_maps = []
    for c in range(N_CORES):
        shard = np.ascontiguousarray(char_ids[c * NSH : (c + 1) * NSH])
        in_maps.append({"ids": shard, "gtab": gtab, "iota": iota, "oones": oones})

    kwargs = {}
    if TRACE:
        kwargs = dict(trace=True, trace_cores=list(range(N_CORES)))
    res = run_bass_kernel_spmd(nc, in_maps, core_ids=list(range(N_CORES)), **kwargs)
    LAST_RESULT = res

    out = np.empty((N, F), np.float32)
    for c in range(N_CORES):
        o = res.results[c]["out"]  # [NGROUP, F, GROUP]
        out[c * NSH : (c + 1) * NSH] = o.transpose(0, 2, 1).reshape(NSH, F)
    return out


# revision 11
# speedup vs baseline: 1.2412x; 1.0083x over previous
"""Trainium2 Bass kernel for a char-CNN (embed lookup + conv1d(K=5,pad=2) + bias + maxpool).

Math: out[n, f] = max_w ( b[f] + sum_k sum_d  E[ids[n, w+k-2], d] * Wc[f, d, k] )

Strategy (pure data-parallel over 8 cores, 4096 tokens each):
  * Host-side constant folding (weights only): G[k][v, f] = sum_d E[v, d] * Wc[f, d, k].
    The embedding+conv collapses to y[n,:,w] = sum_k G[k][ids[n,w+k-2], :] + b.
  * On device, table lookup runs on the TensorEngine as one-hot matmuls with
    contraction over the vocab (96) plus a constant-ones row that carries the bias:
      - ids (bf16, exact for 0..95) broadcast across 96 partitions by the
        otherwise-idle GpSimd engine's partition_broadcast custom instruction,
        keeping the TensorEngine free for tap matmuls only
      - one-hot = is_equal(bcast, iota_per_partition) on VectorE, written fp16
        into a padded [vocab+1, tokens, W+4] layout (dense writes); the 5 tap
        reads are shifted views with token boundaries seeing zeros
      - 5 taps x 2 precision splits PSUM-accumulated against G tables stored as
        fp16 hi + lo (hi+lo recovers ~22 mantissa bits); fp16 weights padded to
        128 columns keep LDWEIGHTS (fast-weight-load) hidden under the matmuls
      - reduce_max over the 16 positions on VectorE (PSUM is DVE-only)
  * The broadcast/one-hot for unit u+1 is emitted before unit u's taps so the
    in-order PE queue never stalls on the VectorE.
  * Output is produced as [group, F, 512] per core; host transposes/concats.
"""

import numpy as np

import concourse.bass as bass
import concourse.bacc as bacc
import concourse.mybir as mybir
from concourse.tile import TileContext
from concourse.bass_utils import run_bass_kernel_spmd

# Problem shapes (hardcoded per contract)
N, W = 32768, 16
VOCAB, D, F, K = 96, 100, 100, 5
N_CORES = 8
NSH = N // N_CORES            # tokens per core = 4096
UNIT = 64                     # tokens per pipeline unit (=> 1024 one-hot cols)
NUNIT = NSH // UNIT           # 64
GROUP = 512                   # tokens per ids DMA
NGROUP = NSH // GROUP         # 8
UPG = GROUP // UNIT           # units per group = 8
VP = VOCAB + 1                # 96 vocab rows + 1 ones row (bias)
WP = W + 4                    # padded char positions per token
FP = 128                      # F padded to 128 weight columns (enables FWL)

bf16 = mybir.dt.bfloat16
f16 = mybir.dt.float16
f32 = mybir.dt.float32
i32 = mybir.dt.int32


def build_nc():
    nc = bacc.Bacc("TRN2", target_bir_lowering=False)

    ids_d = nc.dram_tensor("ids", [NSH, W], i32, kind="ExternalInput")
    # G split tables: [v, (split s, tap k), f_padded]  s=0 -> fp16(G), s=1 -> fp16(G - hi)
    gtab_d = nc.dram_tensor("gtab", [VP, 2 * K, FP], f16, kind="ExternalInput")
    iota_d = nc.dram_tensor("iota", [VOCAB, 1], f32, kind="ExternalInput")
    oones_d = nc.dram_tensor("oones", [1, UNIT * WP], f16, kind="ExternalInput")
    out_d = nc.dram_tensor("out", [NGROUP, F, GROUP], f32, kind="ExternalOutput")

    with TileContext(nc) as tc:
        with (
            tc.tile_pool(name="consts", bufs=1) as consts,
            tc.tile_pool(name="outp", bufs=2) as outp,
            tc.tile_pool(name="idsp", bufs=3) as idsp,
            tc.tile_pool(name="bcp", bufs=3) as bcp,
            tc.tile_pool(name="psA", bufs=1, space="PSUM") as psA,
            tc.tile_pool(name="psB", bufs=3, space="PSUM") as psB,
        ):
            iota_t = consts.tile([VOCAB, 1], f32)
            nc.gpsimd.dma_start(out=iota_t, in_=iota_d[:, :])
            # touch DVE with its steady-state opcodes early: absorbs the
            # engine's first-dispatch latency during the init phase.
            dve_warm = consts.tile([VOCAB, 2], f32, tag="dve_warm")
            nc.vector.tensor_scalar(
                out=dve_warm[:, 0:1],
                in0=iota_t[:, :],
                scalar1=iota_t[:, 0:1],
                scalar2=None,
                op0=mybir.AluOpType.is_equal,
            )
            nc.vector.reduce_max(
                out=dve_warm[:, 1:2],
                in_=iota_t[:, :],
                axis=mybir.AxisListType.X,
            )

            # Two persistent one-hot tiles, padded layout [VP, UNIT, W+4]:
            # char position w at column w+2, pad columns {0,1,18,19} stay zero,
            # row 96 constant 1.0 (bias row, consumed only by center tap).
            o_tiles = []
            s_tiles = []
            for j in range(3):
                ot = consts.tile([VP, UNIT, WP], f16, tag=f"onehot{j}")
                # zero only the pad columns once; is_equal rewrites the real
                # positions every unit, pads stay zero forever.
                nc.vector.memset(ot[0:VOCAB, :, 0:2], 0.0)
                nc.vector.memset(ot[0:VOCAB, :, 2 + W : WP], 0.0)
                nc.gpsimd.dma_start(
                    out=ot[VOCAB : VOCAB + 1, :, :].rearrange("v t w -> v (t w)"),
                    in_=oones_d[:, :],
                )
                o_tiles.append(ot)
                # shifted copy (w+1), maintained by the idle Scalar engine so
                # odd taps read 4-byte-aligned fp16 offsets
                st = consts.tile([VP, UNIT, WP], f16, tag=f"oshift{j}")
                s_tiles.append(st)

            ids_tiles = {}

            def load_ids(g):
                # all of a group's ids in partition 0, converted to bf16
                # (0..95 are exact in bf16)
                idst = idsp.tile([1, GROUP * W], bf16, tag="ids")
                nc.gpsimd.dma_start(
                    out=idst[0:1, :],
                    in_=ids_d[g * GROUP : (g + 1) * GROUP, :].rearrange(
                        "t w -> (t w)"
                    ).unsqueeze(0),
                )
                ids_tiles[g] = idst

            def bcast(u):
                # broadcast ids across 96 partitions on GpSimd + one-hot on DVE
                g, uu = divmod(u, UPG)
                idst = ids_tiles[g]
                bcz = bcp.tile([VOCAB, UNIT, W], bf16, tag="bcast")
                nc.gpsimd.partition_broadcast(
                    out_ap=bcz.rearrange("v t w -> v (t w)"),
                    in_ap=idst[0:1, uu * (UNIT * W) : (uu + 1) * (UNIT * W)],
                    channels=VOCAB,
                )
                # one-hot: O[v, t, w+2] = (ids[t, w] == v), dense write
                o_t = o_tiles[u % 3]
                nc.vector.tensor_scalar(
                    out=o_t[0:VOCAB, :, 2 : 2 + W],
                    in0=bcz[:, :, :],
                    scalar1=iota_t[:, 0:1],
                    scalar2=None,
                    op0=mybir.AluOpType.is_equal,
                )
                # shift-by-one copy for the odd taps (Scalar engine, off the
                # critical path)
                o_s = s_tiles[u % 3]
                nc.scalar.copy(
                    out=o_s[:, :, 0 : WP - 1],
                    in_=o_t[:, :, 1:WP],
                )

            gtab = consts.tile([VP, 2 * K, FP], f16)
            nc.gpsimd.dma_start(
                out=gtab.rearrange("v s f -> v (s f)"),
                in_=gtab_d.rearrange("v s f -> v (s f)"),
            )

            load_ids(0)
            load_ids(1)
            bcast(0)
            bcast(1)

            # PE warmup: emitted after the prologue so the PE's activity window
            # stays busy right up to the first real matmul (HAM stays warm).
            warm = psA.tile([1, 1], f32, tag="warm")
            for _ in range(96):
                nc.tensor.matmul(
                    warm[0:1, 0:1],
                    iota_t[0:1, 0:1],
                    iota_t[0:1, 0:1],
                    start=True,
                    stop=True,
                )
            out_sb = None
            for u in range(NUNIT):
                g, uu = divmod(u, UPG)
                if uu == 0:
                    out_sb = outp.tile([F, GROUP], f32, tag="osb")
                    if g + 2 < NGROUP:
                        load_ids(g + 2)
                # emit bcast+one-hot two units ahead of this unit's taps so
                # the in-order PE queue never stalls on GpSimd/DVE latency.
                if u + 2 < NUNIT:
                    bcast(u + 2)

                o_t = o_tiles[u % 3]
                o_s = s_tiles[u % 3]
                # 5 taps x 2 precision splits, PSUM-accumulated (N=512 each).
                # Odd taps read the shifted tile at even (4B-aligned) offsets.
                ys = [psB.tile([FP, 32, W], f32, tag=f"y{h}", name=f"y{h}") for h in range(2)]
                first = True
                for s in range(2):
                    for k in range(K):
                        src_t, kk = (o_t, k) if k % 2 == 0 else (o_s, k - 1)
                        for h in range(2):
                            nc.tensor.matmul(
                                ys[h][:, :, :],
                                gtab[:, s * K + k, :],
                                src_t[:, h * 32 : (h + 1) * 32, kk : kk + W],
                                start=first,
                                stop=(s == 1 and k == K - 1),
                                skip_group_check=True,
                            )
                        first = False

                # max over the 16 char positions (dense innermost reduce)
                for h in range(2):
                    nc.vector.reduce_max(
                        out=out_sb[:, uu * UNIT + h * 32 : uu * UNIT + (h + 1) * 32],
                        in_=ys[h][0:F, :, :],
                        axis=mybir.AxisListType.X,
                    )

                if uu == UPG - 1:
                    # stream this group's result out to DRAM (contiguous block)
                    nc.sync.dma_start(out=out_d[g, :, :], in_=out_sb[:, :])

    nc.compile()
    return nc


def make_consts(embed_table, conv_w, conv_b):
    # G[k][v, f] = sum_d E[v, d] * Wc[f, d, k] in float64, split hi/lo fp16
    G = np.einsum(
        "vd,fdk->kvf", embed_table.astype(np.float64), conv_w.astype(np.float64)
    )
    Gf = np.zeros((K, VP, F), np.float64)
    Gf[:, 0:VOCAB, :] = G
    Gf[2, VOCAB, :] = conv_b.astype(np.float64)  # bias rides center tap
    hi = Gf.astype(np.float32).astype(np.float16)
    lo = (Gf - hi.astype(np.float64)).astype(np.float32).astype(np.float16)
    gtab = np.zeros((VP, 2 * K, FP), np.float16)
    gtab[:, 0:K, 0:F] = np.transpose(hi, (1, 0, 2))
    gtab[:, K : 2 * K, 0:F] = np.transpose(lo, (1, 0, 2))
    iota = np.arange(VOCAB, dtype=np.float32).reshape(VOCAB, 1)
    oones = np.ones((1, UNIT * WP), np.float16)
    return gtab, iota, oones


_NC_CACHE = {}

# Test-harness knobs (ignored by normal kernel() use)
TRACE = False
LAST_RESULT = None


def kernel(char_ids, embed_table, conv_w, conv_b):
    global LAST_RESULT
    char_ids = np.asarray(char_ids)
    gtab, iota, oones = make_consts(
        np.asarray(embed_table), np.asarray(conv_w), np.asarray(conv_b)
    )

    if "nc" not in _NC_CACHE:
        _NC_CACHE["nc"] = build_nc()
    nc = _NC_CACHE["nc"]

    in_maps = []
    for c in range(N_CORES):
        shard = np.ascontiguousarray(char_ids[c * NSH : (c + 1) * NSH])
        in_maps.append({"ids": shard, "gtab": gtab, "iota": iota, "oones": oones})

    kwargs = {}
    if TRACE:
        kwargs = dict(trace=True, trace_cores=list(range(N_CORES)))
    res = run_bass_kernel_spmd(nc, in_maps, core_ids=list(range(N_CORES)), **kwargs)
    LAST_RESULT = res

    out = np.empty((N, F), np.float32)
    for c in range(N_CORES):
        o = res.results[c]["out"]  # [NGROUP, F, GROUP]
        out[c * NSH : (c + 1) * NSH] = o.transpose(0, 2, 1).reshape(NSH, F)
    return out
